# revision 5
# baseline (speedup 1.0000x reference)
"""Trainium2 Bass kernel for nn_LoopWithIf.

The reference loop
    for i in range(32):
        b = 3*a; s = sum(b); a = a+b if s>0 else a-b
collapses algebraically: the gate's sign is deterministic after the first
iteration, and scaling by 4 / -2 is exact in fp32 (powers of two), so
    out = inp * 2**64      if sum(inp) > 0
    out = inp * -(2**63)   otherwise

Kernel structure (single NEFF, SPMD over 8 NeuronCores, 16MB/core kept
SBUF-resident so the data is read from HBM exactly once):

  phase 1   pipelined DMA loads of column-chunks of the [128, 32768] shard
            (7x4096 + 2048 + 1024 + 1024 wide) + per-chunk reduce_sum on DVE;
            the tapering chunk sizes shorten the reduce tail after the last
            byte lands.

  gate      NO ncfw collective (collective_compute costs ~50us of fixed
            dispatch latency in this runtime -- measured on an otherwise
            empty kernel). Instead: an XOR-pattern cross-core gather with
            raw SDMA remote_dma_broadcast.
              - local total is broadcast to all partitions via a ones-matmul
              - core s sends its [128,1] total to peer s^d, writing column d
                of a [128,8] SBUF tile (receiver r's column d holds core
                r^d's total -- every address is compile-time, no core-id
                dependence)
              - descriptors are prepared during the load phase (SWDGE
                prepare-only) and fired by one trigger_dma gated on the
                local total; arrival bumps a hardware semaphore (+2/sender)
              - the consumer reduce carries `._wait_ge(sem, 14)` inside a
                tile_critical block
            then two DVE tensor_scalar ops select 2**64 / -(2**63).

  phase 2   in-place scale by the factor (DVE, exact power-of-two multiply)
            + pipelined stores, smallest chunk first so the store pipeline
            starts ~0.5us after the factor resolves.

Runtime branching (tc.If / value_load) crashes or fails codegen under
this PJRT/axon execution path, so the kernel is straight-line; the
factor select is pure data flow.
"""

import numpy as np

N_CORES = 8
ROWS = 32            # inp.shape[0]
ROWS_PER_CORE = ROWS // N_CORES
P = 128              # SBUF partitions
FREE = ROWS_PER_CORE * 1024 * 1024 // P   # 32768

LOAD_W = [4096] * 7 + [2048, 1024, 1024]           # sum = 32768
STORE_ORDER = [9, 8, 7, 0, 1, 2, 3, 4, 5, 6]       # smallest first

_nc = None  # compiled kernel cache


def _build(n_cores=N_CORES):
    import concourse.bass as bass  # noqa: F401
    import concourse.bacc as bacc
    import concourse.mybir as mybir
    import concourse.tile as tile

    f32 = mybir.dt.float32
    nc = bacc.Bacc(
        "TRN2",
        target_bir_lowering=False,
        debug=False,
        enable_asserts=False,
        num_devices=n_cores,
    )
    inp_d = nc.dram_tensor("inp", [P, FREE], f32, kind="ExternalInput").ap()
    out_d = nc.dram_tensor("out", [P, FREE], f32, kind="ExternalOutput").ap()

    offs = np.concatenate([[0], np.cumsum(LOAD_W)]).tolist()
    nch = len(LOAD_W)

    from concourse.tile_rust import add_dep_helper

    with tile.TileContext(nc) as tc:
        with (
            tc.tile_pool(name="data", bufs=1) as data_pool,
            tc.tile_pool(name="small", bufs=1) as small_pool,
            tc.tile_pool(name="psum", bufs=1, space="PSUM") as psum_pool,
            tc.tile_pool(name="dram", bufs=1, space="DRAM") as dram_pool,
        ):
            # Semaphores start at 0 each execution (same runtime guarantee
            # Tile's own DMA sems rely on); do NOT sem_clear here -- a
            # scheduler-placed clear can race peer arrivals and wipe them.
            gather_sem = nc.alloc_semaphore("xgather")
            send_sem = nc.alloc_semaphore("xsend")

            chunks = [
                data_pool.tile([P, w], f32, name=f"xchunk{i}", tag=f"xchunk{i}")
                for i, w in enumerate(LOAD_W)
            ]
            partials = small_pool.tile([P, nch], f32, name="partials")
            ones = small_pool.tile([P, P], f32, name="ones")
            nc.vector.memset(ones[:], 1.0)
            buf = small_pool.tile([P, n_cores], f32, name="xbuf")
            tloc = small_pool.tile([P, 1], f32, name="tloc")

            # phase 1: pipelined load + per-chunk reduce
            for i, w in enumerate(LOAD_W):
                nc.sync.dma_start(chunks[i][:], inp_d[:, offs[i] : offs[i] + w])
                nc.vector.reduce_sum(
                    partials[:, i : i + 1], chunks[i][:], axis=mybir.AxisListType.X
                )

            # local total, broadcast to all 128 partitions via ones-matmul
            plocal = small_pool.tile([P, 1], f32, name="plocal")
            nc.vector.reduce_sum(plocal[:], partials[:], axis=mybir.AxisListType.X)
            tpsum = psum_pool.tile([P, 1], f32, name="tpsum")
            nc.tensor.matmul(tpsum[:], ones[:], plocal[:])
            nc.vector.tensor_copy(tloc[:], tpsum[:])
            nc.vector.tensor_copy(buf[:, 0:1], tloc[:])  # self column

            # XOR-gather: prepare one single-dest broadcast per peer distance.
            # Sender s's instruction d lands in column d of peer (s xor d);
            # slot d keeps cross-die distances on D2D-capable engines.
            for d in range(1, n_cores):
                rdests = [None] * n_cores
                rdests[d] = (0, d)
                nc.gpsimd.remote_dma_broadcast(
                    out_ap=buf[:, d : d + 1],
                    in_ap=tloc[:],
                    remote_sem=gather_sem,
                    local_sem=send_sem,
                    rdests=rdests,
                )
            trig = nc.gpsimd.trigger_dma(count=None)

            # Dummy collective: a NEFF containing collectives gets a
            # synchronized cross-core launch (without one, launch skew was
            # measured at up to ~8ms, which the gather wait then eats).
            # Ordered after the xgather trigger so its ~50us ncfw dispatch
            # parks gpsimd only during the store phase, off the critical
            # path. Its data is junk and never consumed.
            dcc_in = dram_pool.tile([P, 1], f32, name="dcc_in")
            dcc_out = dram_pool.tile(
                [n_cores * P, 1], f32, name="dcc_out", addr_space="Shared"
            )
            dcc = nc.gpsimd.collective_compute(
                "AllGather",
                mybir.AluOpType.bypass,
                replica_groups=[list(range(n_cores))],
                ins=[dcc_in.opt()],
                outs=[dcc_out.opt()],
            )
            add_dep_helper(
                trig.ins, dcc.ins, False, "dummy cc after xgather trigger"
            )

            # Inside tile_critical, Tile does NOT insert same-engine RAW
            # semaphores (ordering is manual there), so keep ONLY the
            # sem-gated reduce inside; the critical's exit drain guarantees
            # gtot is fully written before anything outside reads it.
            gtot = small_pool.tile([P, 1], f32, name="gtot")
            fac = small_pool.tile([P, 1], f32, name="fac")
            with tc.tile_critical():
                nc.vector.reduce_sum(
                    gtot[:], buf[:], axis=mybir.AxisListType.X
                )._wait_ge(gather_sem, 2 * (n_cores - 1))

            # factor = 1[tot>0] * 3*2^63 - 2^63  ->  2**64 or -(2**63) (exact)
            nc.vector.tensor_scalar(
                fac[:], gtot[:], 0.0, None, mybir.AluOpType.is_gt
            )
            nc.vector.tensor_scalar(
                fac[:],
                fac[:],
                float(3 * 2**63),
                float(-(2**63)),
                mybir.AluOpType.mult,
                mybir.AluOpType.add,
            )

            # phase 2: in-place scale (DVE) + store, smallest chunk first
            for i in STORE_ORDER:
                nc.vector.tensor_scalar_mul(chunks[i][:], chunks[i][:], fac[:])
                nc.sync.dma_start(
                    out_d[:, offs[i] : offs[i] + LOAD_W[i]], chunks[i][:]
                )

    nc.compile()
    return nc


def _run(in_maps, trace=False):
    from concourse.bass_utils import run_bass_kernel_spmd

    global _nc
    if _nc is None:
        _nc = _build()
    return run_bass_kernel_spmd(
        _nc, in_maps, core_ids=list(range(N_CORES)), trace=trace
    )


def _shard(inp):
    return [
        np.ascontiguousarray(
            inp[c * ROWS_PER_CORE : (c + 1) * ROWS_PER_CORE]
        ).reshape(P, FREE)
        for c in range(N_CORES)
    ]


def _unshard(results):
    out = np.empty((ROWS, 1024, 1024), dtype=np.float32)
    for c in range(N_CORES):
        out[c * ROWS_PER_CORE : (c + 1) * ROWS_PER_CORE] = results[c]["out"].reshape(
            ROWS_PER_CORE, 1024, 1024
        )
    return out


def kernel(**inputs):
    inp = np.ascontiguousarray(np.asarray(inputs["inp"], dtype=np.float32))
    res = _run([{"inp": s} for s in _shard(inp)], trace=False)
    return _unshard(res.results)


def run_traced(inputs):
    """Like kernel() but with NTFF profiling; returns (out, exec_time_ns)."""
    inp = np.ascontiguousarray(np.asarray(inputs["inp"], dtype=np.float32))
    res = _run([{"inp": s} for s in _shard(inp)], trace=True)
    return _unshard(res.results), res.exec_time_ns
